# revision 11
# baseline (speedup 1.0000x reference)
"""Trainium2 Bass kernel for nn_DendriticBranchLayer.

rate = alpha * relu(V - Vth)^2,  V = (exc + cur) / (exc + 1 + cond + inh)
  exc = x @ pruned(pre_w_exc, K=32).T        [B, OUT]
  inh = inhibitory_input @ pruned(pre_w_inh, K=16).T
  cur = sum_f branch_input.reshape(B,OUT,4)[...,f] * w_block[:,f]

Strategy: the top-K masked weights depend only on the small weight tensors, so
the dense masked weights are materialized on the host in fp8 (e4m3). The
TensorEngine runs fp8 at 2x the fp16 rate (DoubleRow pumping: two contraction
rows per instruction), and the kernel is tensor-bound at fp16, so fp8 halves
the matmul time AND the matmul-operand DMA bytes. Accuracy: the top-K mask
keeps only the top ~0.8% quantile of uniform(-2.1,-2.0) pre-weights, so the
surviving weights span a ~0.1% range; scaling that sliver to the top of an
e4m3 binade (and folding the inverse scale into the quantization of x /
inhibitory_input, which feed only their own matmul) makes the weight error
~0.2% RMS. End-to-end simulated rel_l2 = 0.95% vs the 2% gate (x-in-fp8 is
the dominant term; branch_input stays fp16 since it feeds the numerator
unreduced). Batch dim is sharded over 8 cores.
On each core: outputs live on PSUM partitions (128 outputs/block), batch on the
free dim, so all per-output constants (1+cond, Vth, sqrt(alpha), w_block) are
per-partition scalars fed straight into fused DVE/ACT ops.

Every DMA is a contiguous [128, F] transfer: the host pre-swizzles all operands
into the exact SBUF tile layouts.
"""

import numpy as np

import concourse.bass as bass
import concourse.mybir as mybir
import concourse.tile as tile
from concourse import bacc
from concourse.bass_utils import run_bass_kernel_spmd

B, OUT, EXC_IN, INH_IN, BF = 8192, 1024, 4096, 2048, 4
K_EXC, K_INH = 32, 16

NCORES = 8
BC = B // NCORES          # batch per core (1024)
P = 128                   # partitions
NB = 4                    # batch sub-blocks per core
BSUB = BC // NB           # 256 batch per sub-block
OB = OUT // P             # 8 output blocks
KE = EXC_IN // P          # 32 contraction chunks (exc)
KI = INH_IN // P          # 16 contraction chunks (inh)
KQ = 8                    # k-chunks in the first xt subtile
KQ2 = 16                  # end of the second xt subtile

# cst column layout: [P, 3*OB + OB*BF]
_C_CP1 = 0                # 1 + cond, per output
_C_VTHN = OB              # -Vth, per output
_C_SA = 2 * OB            # sqrt(alpha), per output
_C_WB = 3 * OB            # w_block[o, ob*BF + f]
_C_COLS = 3 * OB + OB * BF

_CACHE = {}
TRACE = False  # set by test harness to capture an NTFF profile


def _build_program(wb_ones):
    nc = bacc.Bacc("TRN2", target_bir_lowering=False, debug=False)
    f16, f32 = mybir.dt.float16, mybir.dt.float32
    f8 = mybir.dt.float8e4

    wte = nc.declare_dram_parameter("wte", [P, OB, KE, P], f8, isOutput=False)
    wti = nc.declare_dram_parameter("wti", [P, OB, KI, P], f8, isOutput=False)
    xt = nc.declare_dram_parameter("xt", [NB, P, KE, BSUB], f8, isOutput=False)
    iht = nc.declare_dram_parameter("iht", [NB, P, KI, BSUB], f8, isOutput=False)
    brt = nc.declare_dram_parameter("brt", [NB, OB, P, BF, BSUB], f16, isOutput=False)
    cst = nc.declare_dram_parameter("cst", [P, _C_COLS], f32, isOutput=False)
    outt = nc.declare_dram_parameter("outt", [OB, P, NB, BSUB], f32, isOutput=True)

    add = mybir.AluOpType.add
    mult = mybir.AluOpType.mult
    DR = mybir.MatmulPerfMode.DoubleRow
    Relu = mybir.ActivationFunctionType.Relu
    Square = mybir.ActivationFunctionType.Square
    Identity = mybir.ActivationFunctionType.Identity

    with tile.TileContext(nc) as tc:
        with tc.tile_pool(name="wpool", bufs=1) as wpool, \
             tc.tile_pool(name="xpool", bufs=2) as xpool, \
             tc.tile_pool(name="ipool", bufs=2) as ipool, \
             tc.tile_pool(name="brpool", bufs=4) as brpool, \
             tc.tile_pool(name="wk", bufs=3) as wk, \
             tc.tile_pool(name="wk2", bufs=1) as wk2, \
             tc.tile_pool(name="opool", bufs=3) as opool, \
             tc.tile_pool(name="ps_exc", bufs=4, space="PSUM") as ps_exc, \
             tc.tile_pool(name="ps_inh", bufs=4, space="PSUM") as ps_inh:

            cst_s = wpool.tile([P, _C_COLS], f32)
            # per-ob weight tiles, loaded in first-use order (2 blocks ahead)
            # so the first matmuls don't wait on the whole 12MB weight load
            wte_sb, wti_sb = [None] * OB, [None] * OB

            def load_weights(ob):
                if ob >= OB or wte_sb[ob] is not None:
                    return
                we = wpool.tile([P, KE, P], f8, tag=f"wte{ob}")
                nc.sync.dma_start(out=we, in_=wte[:, ob, :, :])
                wte_sb[ob] = we
                wi = wpool.tile([P, KI, P], f8, tag=f"wti{ob}")
                nc.sync.dma_start(out=wi, in_=wti[:, ob, :, :])
                wti_sb[ob] = wi

            # critical lead-in order: wti0, iht0 (small, first matmuls), then
            # wte0, xta0, xtb0
            wi0 = wpool.tile([P, KI, P], f8, tag="wti0")
            nc.sync.dma_start(out=wi0, in_=wti[:, 0, :, :])
            wti_sb[0] = wi0

            xi_tiles = {}

            def load_nb(nb):
                if nb >= NB or nb in xi_tiles:
                    return
                xsa = xpool.tile([P, KQ, BSUB], f8, tag="xta")
                nc.sync.dma_start(out=xsa, in_=xt[nb, :, 0:KQ, :])
                xsb = xpool.tile([P, KE - KQ, BSUB], f8, tag="xtb")
                nc.sync.dma_start(out=xsb, in_=xt[nb, :, KQ:KE, :])
                xs = (xsa, xsb)
                ihs = ipool.tile([P, KI, BSUB], f8, tag="iht")
                nc.sync.dma_start(out=ihs, in_=iht[nb, :, :, :])
                xi_tiles[nb] = (xs, ihs)

            ihs0 = ipool.tile([P, KI, BSUB], f8, tag="iht")
            nc.sync.dma_start(out=ihs0, in_=iht[0, :, :, :])
            we0 = wpool.tile([P, KE, P], f8, tag="wte0")
            nc.sync.dma_start(out=we0, in_=wte[:, 0, :, :])
            wte_sb[0] = we0
            xsa0 = xpool.tile([P, KQ, BSUB], f8, tag="xta")
            nc.sync.dma_start(out=xsa0, in_=xt[0, :, 0:KQ, :])
            xsb0 = xpool.tile([P, KE - KQ, BSUB], f8, tag="xtb")
            nc.sync.dma_start(out=xsb0, in_=xt[0, :, KQ:KE, :])
            xi_tiles[0] = ((xsa0, xsb0), ihs0)
            nc.sync.dma_start(out=cst_s, in_=cst[:, :])

            for nb in range(NB):
                xt_s, iht_s = xi_tiles[nb]

                for ob in range(OB):
                    br_s = brpool.tile([P, BF, BSUB], f16, tag="br")
                    nc.sync.dma_start(out=br_s, in_=brt[nb, ob, :, :, :])
                    if nb == 0:
                        for ahead in (1, 2, 3, 4):
                            load_weights(ob + ahead)
                    if ob == OB - 3:
                        load_nb(nb + 1)

                    exc_ps = ps_exc.tile([P, BSUB], f32, tag="exc")
                    inh_ps = ps_inh.tile([P, BSUB], f32, tag="inh")

                    def emit_inh():
                        for k in range(0, KI, 2):
                            nc.tensor.matmul(
                                inh_ps, wti_sb[ob][:, k:k + 2, :],
                                iht_s[:, k:k + 2, :],
                                start=(k == 0), stop=(k == KI - 2),
                                perf_mode=DR)

                    def emit_exc():
                        xsa, xsb = xt_s
                        for k in range(0, KE, 2):
                            rhs = (xsa[:, k:k + 2, :] if k < KQ
                                   else xsb[:, k - KQ:k - KQ + 2, :])
                            nc.tensor.matmul(
                                exc_ps, wte_sb[ob][:, k:k + 2, :], rhs,
                                start=(k == 0), stop=(k == KE - 2),
                                perf_mode=DR)

                    if nb == 0 and ob == 0:
                        emit_inh()
                        emit_exc()
                    else:
                        emit_exc()
                        emit_inh()

                    def pointwise(pool, c0, w, sfx):
                        cs = slice(c0, c0 + w)
                        # cur = sum_f br[:, f, cs] * w_block[o, f]  (GpSimd: all-SBUF)
                        if wb_ones:
                            t0 = pool.tile([P, w], f32, tag="cur0" + sfx)
                            nc.gpsimd.tensor_add(t0, br_s[:, 0, cs], br_s[:, 1, cs])
                            t1 = pool.tile([P, w], f32, tag="cur1" + sfx)
                            nc.gpsimd.tensor_add(t1, br_s[:, 2, cs], br_s[:, 3, cs])
                            cur = pool.tile([P, w], f32, tag="cur" + sfx)
                            nc.gpsimd.tensor_add(cur, t0, t1)
                        else:
                            cur = pool.tile([P, w], f32, tag="cur" + sfx)
                            nc.gpsimd.tensor_scalar_mul(
                                cur, br_s[:, 0, cs],
                                cst_s[:, _C_WB + ob * BF: _C_WB + ob * BF + 1])
                            for f in range(1, BF):
                                nxt = pool.tile([P, w], f32, tag=f"cur{f % 2}" + sfx)
                                nc.gpsimd.scalar_tensor_tensor(
                                    nxt, br_s[:, f, cs],
                                    cst_s[:, _C_WB + ob * BF + f: _C_WB + ob * BF + f + 1],
                                    cur, op0=mult, op1=add)
                                cur = nxt

                        num = pool.tile([P, w], f32, tag="num" + sfx)
                        nc.vector.tensor_add(num, exc_ps[:, cs], cur)
                        # exc1 = exc + (1 + cond) on ACT (frees DVE; one PSUM read)
                        exc1 = pool.tile([P, w], f32, tag="exc1" + sfx)
                        nc.scalar.activation(
                            exc1, exc_ps[:, cs], Identity,
                            bias=cst_s[:, _C_CP1 + ob: _C_CP1 + ob + 1])
                        den = pool.tile([P, w], f32, tag="den" + sfx)
                        nc.vector.tensor_add(den, exc1, inh_ps[:, cs])
                        rden = pool.tile([P, w], f32, tag="rden" + sfx)
                        nc.vector.reciprocal_approx_fast(rden, den)
                        v = pool.tile([P, w], f32, tag="v" + sfx)
                        nc.vector.tensor_mul(v, num, rden)
                        # r = relu(v - Vth); rate = (r * sqrt(alpha))^2
                        r = pool.tile([P, w], f32, tag="r" + sfx)
                        nc.scalar.activation(
                            r, v, Relu, bias=cst_s[:, _C_VTHN + ob: _C_VTHN + ob + 1])
                        ot = pool.tile([P, w], f32, tag="ot" + sfx)
                        nc.scalar.activation(
                            ot, r, Square, scale=cst_s[:, _C_SA + ob: _C_SA + ob + 1])
                        nc.sync.dma_start(out=outt[ob, :, nb, cs], in_=ot)

                    if nb == NB - 1 and ob == OB - 1:
                        # split the final chain so the kernel tail is shorter
                        pointwise(wk2, 0, BSUB // 2, "h0")
                        pointwise(wk2, BSUB // 2, BSUB // 2, "h1")
                    else:
                        pointwise(wk, 0, BSUB, "")

    nc.compile()
    return nc


import ml_dtypes

F8 = ml_dtypes.float8_e4m3  # TRN fp8e4 (max 240)


def _pruned_dense_T(pre_w, K):
    """Masked weight, transposed to [in, out], quantized to fp8 e4m3 with the
    sliver scale s (surviving top-K weights span a ~0.1% range; s places them
    at the top of a binade). Returns (W8 [in,out], s): W8 holds s*W; the
    caller quantizes the matching activation as e4m3(act/s) so the fp32 PSUM
    accumulates the unscaled product. Tie-break matches jax.lax.top_k: equal
    values -> lower index wins (stable sort)."""
    idx = np.argsort(-pre_w, axis=1, kind="stable")[:, :K]
    w = np.exp(pre_w.astype(np.float32))
    kept = np.take_along_axis(w, idx, axis=1)
    s = np.float32(0.25 / kept.max() * (1.0 - 2.0 ** -9))
    dense = np.zeros(pre_w.shape, dtype=np.float32)
    np.put_along_axis(dense, idx, kept * s, axis=1)
    return dense.T.astype(F8), s


def kernel(x, inhibitory_input, branch_input, pre_w_exc, pre_w_inh,
           w_block, presigmoid_Vth, log_alpha_max):
    w_block = np.asarray(w_block, dtype=np.float32)
    wb_ones = bool(np.all(w_block == 1.0))
    key = ("nc", wb_ones)
    if key not in _CACHE:
        _CACHE[key] = _build_program(wb_ones)
    nc = _CACHE[key]

    x = np.ascontiguousarray(np.asarray(x, dtype=np.float32))
    inh = np.ascontiguousarray(np.asarray(inhibitory_input, dtype=np.float32))
    br = np.ascontiguousarray(np.asarray(branch_input, dtype=np.float32))
    pre_w_exc = np.asarray(pre_w_exc, dtype=np.float32)
    pre_w_inh = np.asarray(pre_w_inh, dtype=np.float32)
    w_block = np.asarray(w_block, dtype=np.float32)
    presigmoid_Vth = np.asarray(presigmoid_Vth, dtype=np.float32)
    log_alpha_max = np.asarray(log_alpha_max, dtype=np.float32)

    # --- replicated operands -------------------------------------------------
    # wte[p, ob, k, o] = s_e * W_exc[ob*P + o, k*P + p]  (fp8)
    we_t, s_e = _pruned_dense_T(pre_w_exc, K_EXC)     # [EXC_IN, OUT] fp8
    wi_t, s_i = _pruned_dense_T(pre_w_inh, K_INH)     # [INH_IN, OUT] fp8
    wte = np.ascontiguousarray(
        we_t.reshape(KE, P, OB, P).transpose(1, 2, 0, 3))
    wti = np.ascontiguousarray(
        wi_t.reshape(KI, P, OB, P).transpose(1, 2, 0, 3))

    cond = w_block.sum(axis=1, dtype=np.float32)              # [OUT]
    vth = (1.0 / (1.0 + np.exp(-presigmoid_Vth.astype(np.float64)))).astype(np.float32)
    sa = np.sqrt(np.exp(log_alpha_max.astype(np.float32)))
    cst = np.zeros((P, _C_COLS), dtype=np.float32)
    cst[:, _C_CP1:_C_CP1 + OB] = (1.0 + cond).reshape(OB, P).T
    cst[:, _C_VTHN:_C_VTHN + OB] = (-vth).reshape(OB, P).T
    cst[:, _C_SA:_C_SA + OB] = sa.reshape(OB, P).T
    cst[:, _C_WB:] = w_block.reshape(OB, P, BF).transpose(1, 0, 2).reshape(P, OB * BF)

    # --- per-core shards -----------------------------------------------------
    rs_e = np.float32(1.0) / s_e
    rs_i = np.float32(1.0) / s_i
    in_maps = []
    for c in range(NCORES):
        s = slice(c * BC, (c + 1) * BC)
        # xt[nb, p, k, b] = fp8(x[c*BC + nb*BSUB + b, k*P + p] / s_e)
        xt = np.ascontiguousarray(
            (x[s] * rs_e).astype(F8).reshape(NB, BSUB, KE, P).transpose(0, 3, 2, 1))
        iht = np.ascontiguousarray(
            (inh[s] * rs_i).astype(F8).reshape(NB, BSUB, KI, P).transpose(0, 3, 2, 1))
        # brt[nb, ob, o, f, b] = branch[c*BC + nb*BSUB + b, (ob*P + o)*BF + f]
        brt = np.ascontiguousarray(
            br[s].astype(np.float16).reshape(NB, BSUB, OB, P, BF).transpose(0, 2, 3, 4, 1))
        in_maps.append({"wte": wte, "wti": wti, "cst": cst,
                        "xt": xt, "iht": iht, "brt": brt})

    try:
        res = run_bass_kernel_spmd(nc, in_maps, list(range(NCORES)), trace=TRACE)
    except Exception:
        if not TRACE:
            raise
        res = run_bass_kernel_spmd(nc, in_maps, list(range(NCORES)), trace=False)
    _CACHE["last"] = res

    out = np.empty((B, OUT), dtype=np.float32)
    for c in range(NCORES):
        # outt[ob, o, nb, b] -> out[c*BC + nb*BSUB + b, ob*P + o]
        ot = res.results[c]["outt"]
        out[c * BC:(c + 1) * BC] = ot.transpose(2, 3, 0, 1).reshape(BC, OUT)
    return out



# revision 14
# speedup vs baseline: 1.0085x; 1.0085x over previous
"""Trainium2 Bass kernel for nn_DendriticBranchLayer.

rate = alpha * relu(V - Vth)^2,  V = (exc + cur) / (exc + 1 + cond + inh)
  exc = x @ pruned(pre_w_exc, K=32).T        [B, OUT]
  inh = inhibitory_input @ pruned(pre_w_inh, K=16).T
  cur = sum_f branch_input.reshape(B,OUT,4)[...,f] * w_block[:,f]

Strategy: the top-K masked weights depend only on the small weight tensors, so
the dense masked weights are materialized on the host in fp8 (e4m3). The
TensorEngine runs fp8 at 2x the fp16 rate (DoubleRow pumping: two contraction
rows per instruction), and the kernel is tensor-bound at fp16, so fp8 halves
the matmul time AND the matmul-operand DMA bytes. Accuracy: the top-K mask
keeps only the top ~0.8% quantile of uniform(-2.1,-2.0) pre-weights, so the
surviving weights span a ~0.1% range; scaling that sliver to the top of an
e4m3 binade (and folding the inverse scale into the quantization of x /
inhibitory_input, which feed only their own matmul) makes the weight error
~0.2% RMS. End-to-end measured rel_l2 ~= 0.95% vs the 2% gate (x-in-fp8 is
the dominant term; branch_input stays fp16 since it feeds the numerator
unreduced). Batch dim is sharded over 8 cores.

v2 layout (from the v1 fp8 trace: PE active 94us of 132us span; LDWEIGHTS
issue path 108us; 14.6us lead-in, 13us tail, 12.5us of mid gaps):
  - BSUB=512 (full PSUM bank) halves the matmul/LDWEIGHTS instruction count;
    the weight-load path (141ns/ld) then stays well under the PE time.
  - All x/inh/weight DMAs are issued upfront in consumption order into
    dedicated SBUF tiles (they fit; only branch_input streams via a rotating
    pool, 4 tiles ahead) so no matmul waits on a late prefetch.
  - The first tile's operands are split into halves so the first matmul
    starts after ~0.6MB of DMA instead of ~1.8MB.
  - Outputs are written fp16 and DMA'd from the ACT queue right after the
    producing instruction (no Sync-queue contention, no cross-engine sem).
  - den = (exc + (1+cond)) + inh fused into one DVE scalar_tensor_tensor.
Every DMA is a contiguous [128, F] transfer: the host pre-swizzles all
operands into the exact SBUF tile layouts.
"""

import numpy as np
import ml_dtypes

import concourse.bass as bass
import concourse.mybir as mybir
import concourse.tile as tile
from concourse import bacc
from concourse.bass_utils import run_bass_kernel_spmd

B, OUT, EXC_IN, INH_IN, BF = 8192, 1024, 4096, 2048, 4
K_EXC, K_INH = 32, 16

NCORES = 8
BC = B // NCORES          # batch per core (1024)
P = 128                   # partitions
NB = 2                    # batch sub-blocks per core
BSUB = BC // NB           # 512 batch per sub-block (one PSUM bank fp32)
OB = OUT // P             # 8 output blocks
KE = EXC_IN // P          # 32 contraction chunks (exc)
KI = INH_IN // P          # 16 contraction chunks (inh)
KQ = 8                    # k-chunks in the first x/wte subtile
KQ2 = 20                  # end of the second x subtile

F8 = ml_dtypes.float8_e4m3  # TRN fp8e4 (max 240)

# cst column layout: [P, 3*OB + OB*BF]
_C_CP1 = 0                # 1 + cond, per output
_C_VTHN = OB              # -Vth, per output
_C_SA = 2 * OB            # sqrt(alpha), per output
_C_WB = 3 * OB            # w_block[o, ob*BF + f]
_C_COLS = 3 * OB + OB * BF

_CACHE = {}
TRACE = False  # set by test harness to capture an NTFF profile


def _build_program(wb_ones):
    nc = bacc.Bacc("TRN2", target_bir_lowering=False, debug=False)
    f16, f32 = mybir.dt.float16, mybir.dt.float32
    f8 = mybir.dt.float8e4

    wte = nc.declare_dram_parameter("wte", [P, OB, KE, P], f8, isOutput=False)
    wti = nc.declare_dram_parameter("wti", [P, OB, KI, P], f8, isOutput=False)
    xt = nc.declare_dram_parameter("xt", [NB, P, KE, BSUB], f8, isOutput=False)
    iht = nc.declare_dram_parameter("iht", [NB, P, KI, BSUB], f8, isOutput=False)
    brt = nc.declare_dram_parameter("brt", [NB, OB, P, BF, BSUB], f16, isOutput=False)
    cst = nc.declare_dram_parameter("cst", [P, _C_COLS], f32, isOutput=False)
    outt = nc.declare_dram_parameter("outt", [OB, P, NB, BSUB], f16, isOutput=True)

    add = mybir.AluOpType.add
    mult = mybir.AluOpType.mult
    DR = mybir.MatmulPerfMode.DoubleRow
    Relu = mybir.ActivationFunctionType.Relu
    Square = mybir.ActivationFunctionType.Square
    Identity = mybir.ActivationFunctionType.Identity

    with tile.TileContext(nc) as tc:
        with tc.tile_pool(name="data", bufs=1) as dpool, \
             tc.tile_pool(name="brpool", bufs=5) as brpool, \
             tc.tile_pool(name="wk", bufs=2) as wk, \
             tc.tile_pool(name="wk2", bufs=1) as wk2, \
             tc.tile_pool(name="ps_exc", bufs=3, space="PSUM") as ps_exc, \
             tc.tile_pool(name="ps_inh", bufs=3, space="PSUM") as ps_inh:

            def load(shape, dtype, tag, src):
                t = dpool.tile(shape, dtype, tag=tag)
                nc.sync.dma_start(out=t, in_=src)
                return t

            # --- upfront loads, in consumption order --------------------
            # first inh matmuls: k 0..7 of wti[ob=0] and iht[nb=0]
            wti0a = load([P, KQ, P], f8, "wti0a", wti[:, 0, 0:KQ, :])
            ih0a = load([P, KQ, BSUB], f8, "ih0a", iht[0, :, 0:KQ, :])
            wti0b = load([P, KI - KQ, P], f8, "wti0b", wti[:, 0, KQ:KI, :])
            ih0b = load([P, KI - KQ, BSUB], f8, "ih0b", iht[0, :, KQ:KI, :])
            # first exc matmuls
            wte0a = load([P, KQ, P], f8, "wte0a", wte[:, 0, 0:KQ, :])
            xa0 = load([P, KQ, BSUB], f8, "xa0", xt[0, :, 0:KQ, :])
            # first pointwise needs br(0,0) + cst
            br00 = brpool.tile([P, BF, BSUB], f16, tag="br")
            nc.sync.dma_start(out=br00, in_=brt[0, 0, :, :, :])
            cst_s = load([P, _C_COLS], f32, "cst", cst[:, :])
            wte0b = load([P, KE - KQ, P], f8, "wte0b", wte[:, 0, KQ:KE, :])
            xb0a = load([P, KQ2 - KQ, BSUB], f8, "xb0a", xt[0, :, KQ:KQ2, :])
            xb0b = load([P, KE - KQ2, BSUB], f8, "xb0b", xt[0, :, KQ2:KE, :])

            wte_sb, wti_sb = [None] * OB, [None] * OB
            for ob in (1, 2):
                wte_sb[ob] = load([P, KE, P], f8, f"wte{ob}", wte[:, ob, :, :])
                wti_sb[ob] = load([P, KI, P], f8, f"wti{ob}", wti[:, ob, :, :])
            # nb=1 inputs
            xa1 = load([P, KQ, BSUB], f8, "xa1", xt[1, :, 0:KQ, :])
            xb1a = load([P, KQ2 - KQ, BSUB], f8, "xb1a", xt[1, :, KQ:KQ2, :])
            xb1b = load([P, KE - KQ2, BSUB], f8, "xb1b", xt[1, :, KQ2:KE, :])
            ih1 = load([P, KI, BSUB], f8, "ih1", iht[1, :, :, :])
            for ob in range(3, OB):
                wte_sb[ob] = load([P, KE, P], f8, f"wte{ob}", wte[:, ob, :, :])
                wti_sb[ob] = load([P, KI, P], f8, f"wti{ob}", wti[:, ob, :, :])

            xs = {0: (xa0, xb0a, xb0b), 1: (xa1, xb1a, xb1b)}

            def x_sl(nb, k):
                a, b1, b2 = xs[nb]
                if k < KQ:
                    return a[:, k:k + 2, :]
                if k < KQ2:
                    return b1[:, k - KQ:k - KQ + 2, :]
                return b2[:, k - KQ2:k - KQ2 + 2, :]

            def ih_sl(nb, k):
                if nb == 0:
                    if k < KQ:
                        return ih0a[:, k:k + 2, :]
                    return ih0b[:, k - KQ:k - KQ + 2, :]
                return ih1[:, k:k + 2, :]

            def wte_sl(ob, k):
                if ob == 0:
                    if k < KQ:
                        return wte0a[:, k:k + 2, :]
                    return wte0b[:, k - KQ:k - KQ + 2, :]
                return wte_sb[ob][:, k:k + 2, :]

            def wti_sl(ob, k):
                if ob == 0:
                    if k < KQ:
                        return wti0a[:, k:k + 2, :]
                    return wti0b[:, k - KQ:k - KQ + 2, :]
                return wti_sb[ob][:, k:k + 2, :]

            br_tiles = {(0, 0): br00}

            def load_br(t):
                nb, ob = divmod(t, OB)
                if nb >= NB or (nb, ob) in br_tiles:
                    return
                s = brpool.tile([P, BF, BSUB], f16, tag="br")
                nc.sync.dma_start(out=s, in_=brt[nb, ob, :, :, :])
                br_tiles[(nb, ob)] = s

            for t in range(1, 4):
                load_br(t)

            for nb in range(NB):
                for ob in range(OB):
                    load_br(nb * OB + ob + 4)
                    br_s = br_tiles[(nb, ob)]
                    exc_ps = ps_exc.tile([P, BSUB], f32, tag="exc")
                    inh_ps = ps_inh.tile([P, BSUB], f32, tag="inh")

                    def emit_inh():
                        for k in range(0, KI, 2):
                            nc.tensor.matmul(
                                inh_ps, wti_sl(ob, k), ih_sl(nb, k),
                                start=(k == 0), stop=(k == KI - 2),
                                perf_mode=DR)

                    def emit_exc():
                        for k in range(0, KE, 2):
                            nc.tensor.matmul(
                                exc_ps, wte_sl(ob, k), x_sl(nb, k),
                                start=(k == 0), stop=(k == KE - 2),
                                perf_mode=DR)

                    if nb == 0 and ob == 0:
                        emit_inh()
                        emit_exc()
                    else:
                        emit_exc()
                        emit_inh()

                    def pointwise(pool, c0, w, sfx):
                        cs = slice(c0, c0 + w)
                        # cur = sum_f br[:, f, cs] * w_block[o, f]  (GpSimd)
                        if wb_ones:
                            t0 = pool.tile([P, w], f32, tag="cur0" + sfx)
                            nc.gpsimd.tensor_add(t0, br_s[:, 0, cs], br_s[:, 1, cs])
                            t1 = pool.tile([P, w], f32, tag="cur1" + sfx)
                            nc.gpsimd.tensor_add(t1, br_s[:, 2, cs], br_s[:, 3, cs])
                            cur = pool.tile([P, w], f32, tag="cur" + sfx)
                            nc.gpsimd.tensor_add(cur, t0, t1)
                        else:
                            cur = pool.tile([P, w], f32, tag="cur" + sfx)
                            nc.gpsimd.tensor_scalar_mul(
                                cur, br_s[:, 0, cs],
                                cst_s[:, _C_WB + ob * BF: _C_WB + ob * BF + 1])
                            for f in range(1, BF):
                                nxt = pool.tile([P, w], f32, tag=f"cur{f % 2}" + sfx)
                                nc.gpsimd.scalar_tensor_tensor(
                                    nxt, br_s[:, f, cs],
                                    cst_s[:, _C_WB + ob * BF + f: _C_WB + ob * BF + f + 1],
                                    cur, op0=mult, op1=add)
                                cur = nxt

                        num = pool.tile([P, w], f32, tag="num" + sfx)
                        nc.vector.tensor_add(num, exc_ps[:, cs], cur)
                        # exc1 = exc + (1 + cond) on ACT (an op can read only
                        # one PSUM input, so the den add is split)
                        exc1 = pool.tile([P, w], f32, tag="exc1" + sfx)
                        nc.scalar.activation(
                            exc1, exc_ps[:, cs], Identity,
                            bias=cst_s[:, _C_CP1 + ob: _C_CP1 + ob + 1])
                        den = pool.tile([P, w], f32, tag="den" + sfx)
                        nc.vector.tensor_add(den, exc1, inh_ps[:, cs])
                        rden = pool.tile([P, w], f32, tag="rden" + sfx)
                        nc.vector.reciprocal_approx_fast(rden, den)
                        v = pool.tile([P, w], f32, tag="v" + sfx)
                        nc.vector.tensor_mul(v, num, rden)
                        # r = relu(v - Vth); rate = (r * sqrt(alpha))^2
                        r = pool.tile([P, w], f32, tag="r" + sfx)
                        nc.scalar.activation(
                            r, v, Relu, bias=cst_s[:, _C_VTHN + ob: _C_VTHN + ob + 1])
                        ot = pool.tile([P, w], f16, tag="ot" + sfx)
                        nc.scalar.activation(
                            ot, r, Square, scale=cst_s[:, _C_SA + ob: _C_SA + ob + 1])
                        # out DMA from the ACT queue: follows the producer
                        # in queue order, no cross-engine semaphore
                        nc.scalar.dma_start(out=outt[ob, :, nb, cs], in_=ot)

                    if nb == NB - 1 and ob == OB - 1:
                        # split the final chain so the kernel tail is shorter
                        pointwise(wk2, 0, BSUB // 2, "h0")
                        pointwise(wk2, BSUB // 2, BSUB // 2, "h1")
                    else:
                        pointwise(wk, 0, BSUB, "")

    nc.compile()
    return nc


def _pruned_dense_T(pre_w, K):
    """Masked weight, transposed to [in, out], quantized to fp8 e4m3 with the
    sliver scale s (surviving top-K weights span a ~0.1% range; s places them
    at the top of a binade). Returns (W8 [in,out], s): W8 holds s*W; the
    caller quantizes the matching activation as e4m3(act/s) so the fp32 PSUM
    accumulates the unscaled product. Tie-break matches jax.lax.top_k: equal
    values -> lower index wins (stable sort)."""
    idx = np.argsort(-pre_w, axis=1, kind="stable")[:, :K]
    w = np.exp(pre_w.astype(np.float32))
    kept = np.take_along_axis(w, idx, axis=1)
    s = np.float32(0.25 / kept.max() * (1.0 - 2.0 ** -9))
    dense = np.zeros(pre_w.shape, dtype=np.float32)
    np.put_along_axis(dense, idx, kept * s, axis=1)
    return dense.T.astype(F8), s


def kernel(x, inhibitory_input, branch_input, pre_w_exc, pre_w_inh,
           w_block, presigmoid_Vth, log_alpha_max):
    w_block = np.asarray(w_block, dtype=np.float32)
    wb_ones = bool(np.all(w_block == 1.0))
    key = ("nc", wb_ones)
    if key not in _CACHE:
        _CACHE[key] = _build_program(wb_ones)
    nc = _CACHE[key]

    x = np.ascontiguousarray(np.asarray(x, dtype=np.float32))
    inh = np.ascontiguousarray(np.asarray(inhibitory_input, dtype=np.float32))
    br = np.ascontiguousarray(np.asarray(branch_input, dtype=np.float32))
    pre_w_exc = np.asarray(pre_w_exc, dtype=np.float32)
    pre_w_inh = np.asarray(pre_w_inh, dtype=np.float32)
    presigmoid_Vth = np.asarray(presigmoid_Vth, dtype=np.float32)
    log_alpha_max = np.asarray(log_alpha_max, dtype=np.float32)

    # --- replicated operands -------------------------------------------------
    # wte[p, ob, k, o] = s_e * W_exc[ob*P + o, k*P + p]  (fp8)
    we_t, s_e = _pruned_dense_T(pre_w_exc, K_EXC)     # [EXC_IN, OUT] fp8
    wi_t, s_i = _pruned_dense_T(pre_w_inh, K_INH)     # [INH_IN, OUT] fp8
    wte = np.ascontiguousarray(
        we_t.reshape(KE, P, OB, P).transpose(1, 2, 0, 3))
    wti = np.ascontiguousarray(
        wi_t.reshape(KI, P, OB, P).transpose(1, 2, 0, 3))

    cond = w_block.sum(axis=1, dtype=np.float32)              # [OUT]
    vth = (1.0 / (1.0 + np.exp(-presigmoid_Vth.astype(np.float64)))).astype(np.float32)
    sa = np.sqrt(np.exp(log_alpha_max.astype(np.float32)))
    cst = np.zeros((P, _C_COLS), dtype=np.float32)
    cst[:, _C_CP1:_C_CP1 + OB] = (1.0 + cond).reshape(OB, P).T
    cst[:, _C_VTHN:_C_VTHN + OB] = (-vth).reshape(OB, P).T
    cst[:, _C_SA:_C_SA + OB] = sa.reshape(OB, P).T
    cst[:, _C_WB:] = w_block.reshape(OB, P, BF).transpose(1, 0, 2).reshape(P, OB * BF)

    # --- per-core shards -----------------------------------------------------
    rs_e = np.float32(1.0) / s_e
    rs_i = np.float32(1.0) / s_i
    in_maps = []
    for c in range(NCORES):
        s = slice(c * BC, (c + 1) * BC)
        # xt[nb, p, k, b] = fp8(x[c*BC + nb*BSUB + b, k*P + p] / s_e)
        xt = np.ascontiguousarray(
            (x[s] * rs_e).astype(F8).reshape(NB, BSUB, KE, P).transpose(0, 3, 2, 1))
        iht = np.ascontiguousarray(
            (inh[s] * rs_i).astype(F8).reshape(NB, BSUB, KI, P).transpose(0, 3, 2, 1))
        # brt[nb, ob, o, f, b] = branch[c*BC + nb*BSUB + b, (ob*P + o)*BF + f]
        brt = np.ascontiguousarray(
            br[s].astype(np.float16).reshape(NB, BSUB, OB, P, BF).transpose(0, 2, 3, 4, 1))
        in_maps.append({"wte": wte, "wti": wti, "cst": cst,
                        "xt": xt, "iht": iht, "brt": brt})

    try:
        res = run_bass_kernel_spmd(nc, in_maps, list(range(NCORES)), trace=TRACE)
    except Exception:
        if not TRACE:
            raise
        res = run_bass_kernel_spmd(nc, in_maps, list(range(NCORES)), trace=False)
    _CACHE["last"] = res

    out = np.empty((B, OUT), dtype=np.float32)
    for c in range(NCORES):
        # outt[ob, o, nb, b] -> out[c*BC + nb*BSUB + b, ob*P + o]
        ot = res.results[c]["outt"].astype(np.float32)
        out[c * BC:(c + 1) * BC] = ot.transpose(2, 3, 0, 1).reshape(BC, OUT)
    return out


# revision 20
# speedup vs baseline: 1.1125x; 1.1031x over previous
"""Trainium2 Bass kernel for nn_DendriticBranchLayer.

rate = alpha * relu(V - Vth)^2,  V = (exc + cur) / (exc + 1 + cond + inh)
  exc = x @ pruned(pre_w_exc, K=32).T        [B, OUT]
  inh = inhibitory_input @ pruned(pre_w_inh, K=16).T
  cur = sum_f branch_input.reshape(B,OUT,4)[...,f] * w_block[:,f]

Strategy: the top-K masked weights depend only on the small weight tensors, so
the dense masked weights are materialized on the host in fp8 (e4m3). The
TensorEngine runs fp8 at 2x the fp16 rate (DoubleRow pumping: two contraction
rows per instruction), and the kernel is tensor-bound at fp16, so fp8 halves
the matmul time AND the matmul-operand DMA bytes. Accuracy: the top-K mask
keeps only the top ~0.8% quantile of uniform(-2.1,-2.0) pre-weights, so the
surviving weights span a ~0.1% range; scaling that sliver to the top of an
e4m3 binade (and folding the inverse scale into the quantization of x /
inhibitory_input, which feed only their own matmul) makes the weight error
~0.2% RMS. End-to-end measured rel_l2 ~= 0.95% vs the 2% gate (x-in-fp8 is
the dominant term; branch_input stays fp16 since it feeds the numerator
unreduced). Batch dim is sharded over 8 cores.

v2 layout (from the v1 fp8 trace: PE active 94us of 132us span; LDWEIGHTS
issue path 108us; 14.6us lead-in, 13us tail, 12.5us of mid gaps):
  - BSUB=512 (full PSUM bank) halves the matmul/LDWEIGHTS instruction count;
    the weight-load path (141ns/ld) then stays well under the PE time.
  - All x/inh/weight DMAs are issued upfront in consumption order into
    dedicated SBUF tiles (they fit; only branch_input streams via a rotating
    pool, 4 tiles ahead) so no matmul waits on a late prefetch.
  - The first tile's operands are split into halves so the first matmul
    starts after ~0.6MB of DMA instead of ~1.8MB.
  - Outputs are written fp16 and DMA'd from the ACT queue right after the
    producing instruction (no Sync-queue contention, no cross-engine sem).
Every DMA is a contiguous [128, F] transfer: the host pre-swizzles all
operands into the exact SBUF tile layouts.
"""

import numpy as np
import ml_dtypes

import concourse.bass as bass
import concourse.mybir as mybir
import concourse.tile as tile
from concourse import bacc
from concourse.bass_utils import run_bass_kernel_spmd

B, OUT, EXC_IN, INH_IN, BF = 8192, 1024, 4096, 2048, 4
K_EXC, K_INH = 32, 16

NCORES = 8
BC = B // NCORES          # batch per core (1024)
P = 128                   # partitions
NB = 2                    # batch sub-blocks per core
BSUB = BC // NB           # 512 batch per sub-block (one PSUM bank fp32)
OB = OUT // P             # 8 output blocks
KE = EXC_IN // P          # 32 contraction chunks (exc)
KI = INH_IN // P          # 16 contraction chunks (inh)
KQ = 8                    # k-chunks in the first x/wte subtile
KQ2 = 20                  # end of the second x subtile

F8 = ml_dtypes.float8_e4m3  # TRN fp8e4 (max 240)

# cst column layout: [P, 3*OB + OB*BF]
_C_CP1 = 0                # 1 + cond, per output
_C_VTHN = OB              # -Vth, per output
_C_SA = 2 * OB            # sqrt(alpha), per output
_C_WB = 3 * OB            # w_block[o, ob*BF + f]
_C_COLS = 3 * OB + OB * BF

_CACHE = {}
TRACE = False  # set by test harness to capture an NTFF profile


def _build_program(wb_ones):
    nc = bacc.Bacc("TRN2", target_bir_lowering=False, debug=False)
    f16, f32 = mybir.dt.float16, mybir.dt.float32
    f8 = mybir.dt.float8e4

    wte = nc.declare_dram_parameter("wte", [P, OB, KE, P], f8, isOutput=False)
    wti = nc.declare_dram_parameter("wti", [P, OB, KI, P], f8, isOutput=False)
    xt = nc.declare_dram_parameter("xt", [NB, P, KE, BSUB], f8, isOutput=False)
    iht = nc.declare_dram_parameter("iht", [NB, P, KI, BSUB], f8, isOutput=False)
    brt = nc.declare_dram_parameter("brt", [NB, OB, P, BF, BSUB], f16, isOutput=False)
    cst = nc.declare_dram_parameter("cst", [P, _C_COLS], f32, isOutput=False)
    outt = nc.declare_dram_parameter("outt", [OB, P, NB, BSUB], f16, isOutput=True)

    add = mybir.AluOpType.add
    mult = mybir.AluOpType.mult
    DR = mybir.MatmulPerfMode.DoubleRow
    Relu = mybir.ActivationFunctionType.Relu
    Square = mybir.ActivationFunctionType.Square
    Identity = mybir.ActivationFunctionType.Identity

    with tile.TileContext(nc) as tc:
        with tc.tile_pool(name="data", bufs=1) as dpool, \
             tc.tile_pool(name="brpool", bufs=8) as brpool, \
             tc.tile_pool(name="wk", bufs=2) as wk, \
             tc.tile_pool(name="wk2", bufs=1) as wk2, \
             tc.tile_pool(name="ps_exc", bufs=4, space="PSUM") as ps_exc, \
             tc.tile_pool(name="ps_inh", bufs=3, space="PSUM") as ps_inh:

            def load(shape, dtype, tag, src, eng=None):
                t = dpool.tile(shape, dtype, tag=tag)
                (eng or nc.sync).dma_start(out=t, in_=src)
                return t

            # --- upfront loads, in strict consumption order -------------
            # The DMA engines drain the queue in issue order at ~330GB/s,
            # so anything issued before data needed earlier DELAYS it.
            # First-tile pieces go on the Scalar queue (the only other
            # HW-DGE engine): it clears its preamble ~1.7us before Sync.
            KA = 4   # first-piece k-chunks (smallest useful lead-in)
            wti0a = load([P, KA, P], f8, "wti0a", wti[:, 0, 0:KA, :], nc.scalar)
            ih0a = load([P, KA, BSUB], f8, "ih0a", iht[0, :, 0:KA, :], nc.scalar)
            wti0b = load([P, KI - KA, P], f8, "wti0b", wti[:, 0, KA:KI, :], nc.scalar)
            ih0b = load([P, KI - KA, BSUB], f8, "ih0b", iht[0, :, KA:KI, :], nc.scalar)
            wte0a = load([P, KQ, P], f8, "wte0a", wte[:, 0, 0:KQ, :], nc.scalar)
            xa0 = load([P, KQ, BSUB], f8, "xa0", xt[0, :, 0:KQ, :], nc.scalar)
            # rest of tile 0 on Sync
            wte0b = load([P, KE - KQ, P], f8, "wte0b", wte[:, 0, KQ:KE, :])
            xb0a = load([P, KQ2 - KQ, BSUB], f8, "xb0a", xt[0, :, KQ:KQ2, :])
            xb0b = load([P, KE - KQ2, BSUB], f8, "xb0b", xt[0, :, KQ2:KE, :])
            # first pointwise needs br(0,0) + cst
            br00 = brpool.tile([P, BF, BSUB], f16, tag="br")
            nc.sync.dma_start(out=br00, in_=brt[0, 0, :, :, :])
            cst_s = load([P, _C_COLS], f32, "cst", cst[:, :])

            # per-tile groups in consumption order: weights(ob) + br(0,ob);
            # nb=1 x/inh interleaved where first needed (~tile 8)
            wte_sb, wti_sb = [None] * OB, [None] * OB
            br_tiles = {(0, 0): br00}

            def load_w(ob):
                wte_sb[ob] = load([P, KE, P], f8, f"wte{ob}", wte[:, ob, :, :])
                wti_sb[ob] = load([P, KI, P], f8, f"wti{ob}", wti[:, ob, :, :])

            def load_br(t):
                nb, ob = divmod(t, OB)
                if nb >= NB or (nb, ob) in br_tiles:
                    return
                s = brpool.tile([P, BF, BSUB], f16, tag="br")
                nc.sync.dma_start(out=s, in_=brt[nb, ob, :, :, :])
                br_tiles[(nb, ob)] = s

            for ob in (1, 2, 3, 4):
                load_w(ob)
                load_br(ob)
            xa1 = load([P, KQ, BSUB], f8, "xa1", xt[1, :, 0:KQ, :])
            xb1a = load([P, KQ2 - KQ, BSUB], f8, "xb1a", xt[1, :, KQ:KQ2, :])
            load_w(5)
            load_br(5)
            xb1b = load([P, KE - KQ2, BSUB], f8, "xb1b", xt[1, :, KQ2:KE, :])
            ih1 = load([P, KI, BSUB], f8, "ih1", iht[1, :, :, :])
            load_w(6)
            load_br(6)
            load_w(7)
            load_br(7)

            xs = {0: (xa0, xb0a, xb0b), 1: (xa1, xb1a, xb1b)}

            def x_sl(nb, k):
                a, b1, b2 = xs[nb]
                if k < KQ:
                    return a[:, k:k + 2, :]
                if k < KQ2:
                    return b1[:, k - KQ:k - KQ + 2, :]
                return b2[:, k - KQ2:k - KQ2 + 2, :]

            def ih_sl(nb, k):
                if nb == 0:
                    if k < KA:
                        return ih0a[:, k:k + 2, :]
                    return ih0b[:, k - KA:k - KA + 2, :]
                return ih1[:, k:k + 2, :]

            def wte_sl(ob, k):
                if ob == 0:
                    if k < KQ:
                        return wte0a[:, k:k + 2, :]
                    return wte0b[:, k - KQ:k - KQ + 2, :]
                return wte_sb[ob][:, k:k + 2, :]

            def wti_sl(ob, k):
                if ob == 0:
                    if k < KA:
                        return wti0a[:, k:k + 2, :]
                    return wti0b[:, k - KA:k - KA + 2, :]
                return wti_sb[ob][:, k:k + 2, :]

            for nb in range(NB):
                for ob in range(OB):
                    load_br(nb * OB + ob + 4)
                    br_s = br_tiles[(nb, ob)]
                    exc_ps = ps_exc.tile([P, BSUB], f32, tag="exc")
                    inh_ps = ps_inh.tile([P, BSUB], f32, tag="inh")

                    def emit_inh():
                        for k in range(0, KI, 2):
                            nc.tensor.matmul(
                                inh_ps, wti_sl(ob, k), ih_sl(nb, k),
                                start=(k == 0), stop=(k == KI - 2),
                                perf_mode=DR)

                    def emit_exc():
                        for k in range(0, KE, 2):
                            nc.tensor.matmul(
                                exc_ps, wte_sl(ob, k), x_sl(nb, k),
                                start=(k == 0), stop=(k == KE - 2),
                                perf_mode=DR)

                    if nb == 0 and ob == 0:
                        emit_inh()
                        emit_exc()
                    else:
                        emit_exc()
                        emit_inh()

                    def pointwise(pool, c0, w, sfx):
                        cs = slice(c0, c0 + w)
                        # cur = sum_f br[:, f, cs] * w_block[o, f]  (GpSimd)
                        if wb_ones:
                            t0 = pool.tile([P, w], f32, tag="cur0" + sfx)
                            nc.gpsimd.tensor_add(t0, br_s[:, 0, cs], br_s[:, 1, cs])
                            t1 = pool.tile([P, w], f32, tag="cur1" + sfx)
                            nc.gpsimd.tensor_add(t1, br_s[:, 2, cs], br_s[:, 3, cs])
                            cur = pool.tile([P, w], f32, tag="cur" + sfx)
                            nc.gpsimd.tensor_add(cur, t0, t1)
                        else:
                            cur = pool.tile([P, w], f32, tag="cur" + sfx)
                            nc.gpsimd.tensor_scalar_mul(
                                cur, br_s[:, 0, cs],
                                cst_s[:, _C_WB + ob * BF: _C_WB + ob * BF + 1])
                            for f in range(1, BF):
                                nxt = pool.tile([P, w], f32, tag=f"cur{f % 2}" + sfx)
                                nc.gpsimd.scalar_tensor_tensor(
                                    nxt, br_s[:, f, cs],
                                    cst_s[:, _C_WB + ob * BF + f: _C_WB + ob * BF + f + 1],
                                    cur, op0=mult, op1=add)
                                cur = nxt

                        num = pool.tile([P, w], f32, tag="num" + sfx)
                        nc.vector.tensor_add(num, exc_ps[:, cs], cur)
                        # exc1 = exc + (1 + cond) on ACT (an op can read only
                        # one PSUM input, so the den add is split)
                        exc1 = pool.tile([P, w], f32, tag="exc1" + sfx)
                        nc.scalar.activation(
                            exc1, exc_ps[:, cs], Identity,
                            bias=cst_s[:, _C_CP1 + ob: _C_CP1 + ob + 1])
                        den = pool.tile([P, w], f32, tag="den" + sfx)
                        nc.vector.tensor_add(den, exc1, inh_ps[:, cs])
                        rden = pool.tile([P, w], f32, tag="rden" + sfx)
                        nc.vector.reciprocal_approx_fast(rden, den)
                        v = pool.tile([P, w], f32, tag="v" + sfx)
                        nc.vector.tensor_mul(v, num, rden)
                        # r = relu(v - Vth); rate = (r * sqrt(alpha))^2
                        r = pool.tile([P, w], f32, tag="r" + sfx)
                        nc.scalar.activation(
                            r, v, Relu, bias=cst_s[:, _C_VTHN + ob: _C_VTHN + ob + 1])
                        ot = pool.tile([P, w], f16, tag="ot" + sfx)
                        nc.scalar.activation(
                            ot, r, Square, scale=cst_s[:, _C_SA + ob: _C_SA + ob + 1])
                        # out DMA from the ACT queue: follows the producer
                        # in queue order, no cross-engine semaphore
                        nc.scalar.dma_start(out=outt[ob, :, nb, cs], in_=ot)

                    if nb == NB - 1 and ob == OB - 1:
                        # split the final chain so the kernel tail is shorter
                        pointwise(wk2, 0, BSUB // 2, "h0")
                        pointwise(wk2, BSUB // 2, BSUB // 2, "h1")
                    else:
                        pointwise(wk, 0, BSUB, "")

    nc.compile()
    return nc


def _pruned_dense_T(pre_w, K):
    """Masked weight, transposed to [in, out], quantized to fp8 e4m3 with the
    sliver scale s (surviving top-K weights span a ~0.1% range; s places them
    at the top of a binade). Returns (W8 [in,out], s): W8 holds s*W; the
    caller quantizes the matching activation as e4m3(act/s) so the fp32 PSUM
    accumulates the unscaled product. Tie-break matches jax.lax.top_k: equal
    values -> lower index wins (stable sort)."""
    idx = np.argsort(-pre_w, axis=1, kind="stable")[:, :K]
    w = np.exp(pre_w.astype(np.float32))
    kept = np.take_along_axis(w, idx, axis=1)
    s = np.float32(0.25 / kept.max() * (1.0 - 2.0 ** -9))
    dense = np.zeros(pre_w.shape, dtype=np.float32)
    np.put_along_axis(dense, idx, kept * s, axis=1)
    return dense.T.astype(F8), s


def kernel(x, inhibitory_input, branch_input, pre_w_exc, pre_w_inh,
           w_block, presigmoid_Vth, log_alpha_max):
    w_block = np.asarray(w_block, dtype=np.float32)
    wb_ones = bool(np.all(w_block == 1.0))
    key = ("nc", wb_ones)
    if key not in _CACHE:
        _CACHE[key] = _build_program(wb_ones)
    nc = _CACHE[key]

    x = np.ascontiguousarray(np.asarray(x, dtype=np.float32))
    inh = np.ascontiguousarray(np.asarray(inhibitory_input, dtype=np.float32))
    br = np.ascontiguousarray(np.asarray(branch_input, dtype=np.float32))
    pre_w_exc = np.asarray(pre_w_exc, dtype=np.float32)
    pre_w_inh = np.asarray(pre_w_inh, dtype=np.float32)
    presigmoid_Vth = np.asarray(presigmoid_Vth, dtype=np.float32)
    log_alpha_max = np.asarray(log_alpha_max, dtype=np.float32)

    # --- replicated operands -------------------------------------------------
    # wte[p, ob, k, o] = s_e * W_exc[ob*P + o, k*P + p]  (fp8)
    we_t, s_e = _pruned_dense_T(pre_w_exc, K_EXC)     # [EXC_IN, OUT] fp8
    wi_t, s_i = _pruned_dense_T(pre_w_inh, K_INH)     # [INH_IN, OUT] fp8
    wte = np.ascontiguousarray(
        we_t.reshape(KE, P, OB, P).transpose(1, 2, 0, 3))
    wti = np.ascontiguousarray(
        wi_t.reshape(KI, P, OB, P).transpose(1, 2, 0, 3))

    cond = w_block.sum(axis=1, dtype=np.float32)              # [OUT]
    vth = (1.0 / (1.0 + np.exp(-presigmoid_Vth.astype(np.float64)))).astype(np.float32)
    sa = np.sqrt(np.exp(log_alpha_max.astype(np.float32)))
    cst = np.zeros((P, _C_COLS), dtype=np.float32)
    cst[:, _C_CP1:_C_CP1 + OB] = (1.0 + cond).reshape(OB, P).T
    cst[:, _C_VTHN:_C_VTHN + OB] = (-vth).reshape(OB, P).T
    cst[:, _C_SA:_C_SA + OB] = sa.reshape(OB, P).T
    cst[:, _C_WB:] = w_block.reshape(OB, P, BF).transpose(1, 0, 2).reshape(P, OB * BF)

    # --- per-core shards -----------------------------------------------------
    rs_e = np.float32(1.0) / s_e
    rs_i = np.float32(1.0) / s_i
    in_maps = []
    for c in range(NCORES):
        s = slice(c * BC, (c + 1) * BC)
        # xt[nb, p, k, b] = fp8(x[c*BC + nb*BSUB + b, k*P + p] / s_e)
        xt = np.ascontiguousarray(
            (x[s] * rs_e).astype(F8).reshape(NB, BSUB, KE, P).transpose(0, 3, 2, 1))
        iht = np.ascontiguousarray(
            (inh[s] * rs_i).astype(F8).reshape(NB, BSUB, KI, P).transpose(0, 3, 2, 1))
        # brt[nb, ob, o, f, b] = branch[c*BC + nb*BSUB + b, (ob*P + o)*BF + f]
        brt = np.ascontiguousarray(
            br[s].astype(np.float16).reshape(NB, BSUB, OB, P, BF).transpose(0, 2, 3, 4, 1))
        in_maps.append({"wte": wte, "wti": wti, "cst": cst,
                        "xt": xt, "iht": iht, "brt": brt})

    try:
        res = run_bass_kernel_spmd(nc, in_maps, list(range(NCORES)), trace=TRACE)
    except Exception:
        if not TRACE:
            raise
        res = run_bass_kernel_spmd(nc, in_maps, list(range(NCORES)), trace=False)
    _CACHE["last"] = res

    out = np.empty((B, OUT), dtype=np.float32)
    for c in range(NCORES):
        # outt[ob, o, nb, b] -> out[c*BC + nb*BSUB + b, ob*P + o]
        ot = res.results[c]["outt"].astype(np.float32)
        out[c * BC:(c + 1) * BC] = ot.transpose(2, 3, 0, 1).reshape(BC, OUT)
    return out
